# revision 8
# baseline (speedup 1.0000x reference)
"""DiffLogic 3-layer network on 8 Trainium2 NeuronCores.

Strategy (data-parallel over batch, per spec hint):
  - Each core gets 512 of the 4096 batch rows. Weights/indices replicated.
  - Activations feature-major ([features, batch] fp16); per-layer feature
    gathers are gpsimd dma_gather row gathers (random 1KB rows DRAM->SBUF).
  - v2 changes vs the first working version:
      * a- and b-gathers for each 8-chunk group are merged into ONE
        dma_gather of 2048 indices (halves the per-gather SWDGE fixed cost
        on the gpsimd engine, which the scheduling-sim showed as the top
        engine), spread round-robin over 4 SWDGE queues.
      * softmax->polynomial coefficients (ck = softmax(w) @ OP_COEF) are
        computed on the host and passed in as small fp32 tensors instead of
        being derived on-device from exp-sums.
      * layer3 + group-sum run as 3 PE matmuls per chunk with
        coefficient-weighted one-hot stationaries (a, b, ab terms); the c0
        term is folded into a single ones-matmul with host-baked per-group
        column sums (W0), removing 80 per-chunk ones-matmuls.
      * elementwise mixture restructured as
            v  = c3*b + c1        (ACT affine)
            t3 = c2*b + c0        (split ACT / DVE tensor_scalar [4x mode])
            w  = v * a            (DVE tensor_tensor)
            h  = w + t3           (DVE tensor_tensor)
        which keeps DVE on its 2x/4x fast paths.
  - Layer output rows are permuted host-side so each layer's a-operand
    gather reads DRAM rows in ascending order (HBM locality).
"""

import numpy as np

# ---- problem constants (hardcoded per contract) ----
B, D0, D1, D2, D3 = 4096, 1024, 8192, 8192, 10240
NCORES = 8
BS = B // NCORES  # 512 batch rows per core
K = 10
TAU = 30.0

_OP_COEF = np.array([
    [0., 0., 0., 0.], [0., 0., 0., 1.], [0., 1., 0., -1.], [0., 1., 0., 0.],
    [0., 0., 1., -1.], [0., 0., 1., 0.], [0., 1., 1., -2.], [0., 1., 1., -1.],
    [1., -1., -1., 1.], [1., -1., -1., 2.], [1., 0., -1., 0.], [1., 0., -1., 1.],
    [1., -1., 0., 0.], [1., -1., 0., 1.], [1., 0., 0., -1.], [1., 0., 0., 0.],
], dtype=np.float64)

LAYERS = [
    # (n_out, n_src)
    (D1, D0),
    (D2, D1),
    (D3, D2),
]
NCH = [o // 128 for o, _ in LAYERS]  # [64, 64, 80]
NCH_TOT = sum(NCH)  # 208
CH_OFF = [0, NCH[0], NCH[0] + NCH[1]]
GRP = 8        # chunks per merged gather (8*128 a-rows + 8*128 b-rows)
TG = 4         # chunks per grouped tensor_tensor
T3_DVE_MOD = 1  # chunk-groups with lc % T3_DVE_MOD == 0 run t3 on DVE (rest ACT)
POOL_ADD_MOD = 3  # tt-groups with tti % POOL_ADD_MOD == 0 run h-add on gpsimd

_nc_cache = {}


def _build_nc(repeat=1, t3_dve_mod=T3_DVE_MOD, pool_add_mod=POOL_ADD_MOD,
              nq=4, grp=GRP, tg=TG):
    from concourse import bacc, bass, mybir
    from concourse.tile import TileContext

    f16 = mybir.dt.float16
    f32 = mybir.dt.float32
    i16 = mybir.dt.int16
    Alu = mybir.AluOpType
    Act = mybir.ActivationFunctionType

    nc = bacc.Bacc(None, target_bir_lowering=False, num_swdge_queues=nq)

    # ---- I/O ----
    xT = nc.dram_tensor("xT", [D0, BS], f16, kind="ExternalInput")
    ck_d = [
        nc.dram_tensor(f"ck{k}", [128, NCH_TOT], f32, kind="ExternalInput")
        for k in range(4)
    ]
    selk_d = [
        nc.dram_tensor(f"selk{k}", [128, NCH[2] * K], f16, kind="ExternalInput")
        for k in ("g", "2")
    ]
    w0_d = nc.dram_tensor("w0sum", [128, K], f16, kind="ExternalInput")
    idx_in = []
    for li, (o, _) in enumerate(LAYERS):
        # merged (a||b per group) wrapped indices: [128, 2*o/16]
        iab = nc.dram_tensor(f"iab{li}", [128, 2 * o // 16], i16, kind="ExternalInput")
        idx_in.append(iab)
    out_d = nc.dram_tensor("out", [K, BS], f32, kind="ExternalOutput")

    # intermediate activations, partition-major: h[p, c, b] = row (c*128+p)
    h_d = [
        nc.dram_tensor("h1", [128, NCH[0], BS], f16),
        nc.dram_tensor("h2", [128, NCH[1], BS], f16),
    ]
    src_ap = [
        lambda: xT[:],
        lambda: h_d[0][:].rearrange("p c b -> (p c) b"),
        lambda: h_d[1][:].rearrange("p c b -> (p c) b"),
    ]

    with TileContext(nc) as tc:
      for _rep in range(repeat):
        with (
            tc.tile_pool(name="pers", bufs=1) as pers,
            tc.tile_pool(name="psum", bufs=1, space="PSUM") as psump,
        ):
            ck = []
            for k in range(4):
                t = pers.tile([128, NCH_TOT], f32, name=f"ck{k}")
                nc.sync.dma_start(out=t[:], in_=ck_d[k][:])
                ck.append(t)
            selk = {}
            for j, k in enumerate(("g", "2")):
                t = pers.tile([128, NCH[2] * K], f16, name=f"selk{k}")
                nc.sync.dma_start(out=t[:], in_=selk_d[j][:])
                selk[k] = t
            w0_t = pers.tile([128, K], f16)
            nc.sync.dma_start(out=w0_t[:], in_=w0_d[:])
            ones_t = pers.tile([128, BS], f16)
            nc.vector.memset(ones_t[:], 1.0)

            psum_out = psump.tile([K, BS], f32, space="PSUM")
            with (
                tc.tile_pool(name="idxp", bufs=2) as idxp,
                tc.tile_pool(name="gath", bufs=3) as gath,
                tc.tile_pool(name="outp", bufs=3) as outp,
                tc.tile_pool(name="tmp", bufs=3) as tmp,
            ):
                # c0-term of layer3 group sums: one matmul, starts the psum
                nc.tensor.matmul(
                    out=psum_out[:], lhsT=w0_t[:], rhs=ones_t[:],
                    start=True, stop=False,
                )
                gq = 0
                for li, (o, n_src) in enumerate(LAYERS):
                    nch = NCH[li]
                    iab_t = idxp.tile([128, 2 * o // 16], i16, tag="iab")
                    nc.sync.dma_start(out=iab_t[:], in_=idx_in[li][:])

                    for g in range(nch // grp):
                        # merged gather: slots 0..grp-1 = a-chunks,
                        # grp..2*grp-1 = b-chunks
                        gAB = gath.tile([128, 2 * grp, BS], f16, tag="gAB")
                        nc.gpsimd.dma_gather(
                            out_ap=gAB[:],
                            in_ap=src_ap[li](),
                            idxs_ap=iab_t[:, g * 2 * grp * 8 : (g + 1) * 2 * grp * 8],
                            num_idxs=2 * grp * 128,
                            num_idxs_reg=2 * grp * 128,
                            elem_size=BS,
                            single_packet=False,
                            queue_num=gq % nq,
                        )
                        gq += 1
                        ho = outp.tile([128, grp, BS], f16, tag="ho")
                        for cg in range(grp // tg):
                            vg = tmp.tile([128, tg, BS], f16, tag="vg")
                            t3g = None
                            if li < 2:
                                t3g = tmp.tile([128, tg, BS], f16,
                                               name="t3g", tag="t3g")
                            for c4 in range(tg):
                                c = cg * tg + c4
                                lc = g * grp + c
                                gc = CH_OFF[li] + lc
                                b = gAB[:, grp + c, :]
                                # v = c3*b + c1  (ACT)
                                nc.scalar.activation(
                                    out=vg[:, c4, :],
                                    in_=b,
                                    func=Act.Identity,
                                    scale=ck[3][:, gc : gc + 1],
                                    bias=ck[1][:, gc : gc + 1],
                                )
                                if li == 2:
                                    continue  # c2/c0 terms via selk2/W0 matmuls
                                # t3 = c2*b + c0  (split DVE ts / ACT)
                                if lc % t3_dve_mod == 0:
                                    nc.vector.tensor_scalar(
                                        out=t3g[:, c4, :],
                                        in0=b,
                                        scalar1=ck[2][:, gc : gc + 1],
                                        scalar2=ck[0][:, gc : gc + 1],
                                        op0=Alu.mult,
                                        op1=Alu.add,
                                    )
                                else:
                                    nc.scalar.activation(
                                        out=t3g[:, c4, :],
                                        in_=b,
                                        func=Act.Identity,
                                        scale=ck[2][:, gc : gc + 1],
                                        bias=ck[0][:, gc : gc + 1],
                                    )
                            # w = v * a (grouped)
                            wg = tmp.tile([128, tg, BS], f16, tag="wg")
                            nc.vector.tensor_tensor(
                                out=wg[:],
                                in0=vg[:],
                                in1=gAB[:, cg * tg : (cg + 1) * tg, :],
                                op=Alu.mult,
                            )
                            if li < 2:
                                # h = w + t3 (a slice of groups on gpsimd to
                                # offload DVE)
                                tti = (CH_OFF[li] * 0 + g * grp + cg * tg) // tg \
                                    + li * (NCH[0] // tg)
                                eng = (
                                    nc.gpsimd
                                    if pool_add_mod and tti % pool_add_mod == 0
                                    else nc.vector
                                )
                                eng.tensor_tensor(
                                    out=ho[:, cg * tg : (cg + 1) * tg, :],
                                    in0=wg[:],
                                    in1=t3g[:],
                                    op=Alu.add,
                                )
                            else:
                                # w = (c3 b + c1) a already combines the
                                # c3*ab and c1*a terms, so the group sum is
                                #   selg @ w  (unweighted group one-hot)
                                # + selk2 @ b (c2 term; c0 via W0 matmul)
                                for c4 in range(tg):
                                    c = cg * tg + c4
                                    lc = g * grp + c
                                    sl = slice(lc * K, (lc + 1) * K)
                                    last = lc == NCH[2] - 1
                                    nc.tensor.matmul(
                                        out=psum_out[:],
                                        lhsT=selk["g"][:, sl],
                                        rhs=wg[:, c4, :],
                                        start=False, stop=False,
                                    )
                                    nc.tensor.matmul(
                                        out=psum_out[:],
                                        lhsT=selk["2"][:, sl],
                                        rhs=gAB[:, grp + c, :],
                                        start=False, stop=last,
                                    )
                        if li < 2:
                            nc.sync.dma_start(
                                out=h_d[li][:, g * grp : (g + 1) * grp, :],
                                in_=ho[:],
                            )

            out_sb = pers.tile([K, BS], f32)
            nc.scalar.activation(
                out=out_sb[:], in_=psum_out[:], func=Act.Copy, scale=1.0 / TAU
            )
            nc.sync.dma_start(out=out_d[:], in_=out_sb[:])

    nc.compile()
    return nc


def _wrap_idx_merged(ia: np.ndarray, ib: np.ndarray, grp: int) -> np.ndarray:
    """Merged (a||b per group) int16 wrapped index layout for dma_gather:
    per group of grp*128 rows, the 2*grp*128 index list is
    [a rows of chunks g*grp..g*grp+grp, b rows of same chunks]; the whole
    list is wrapped in 16 partitions and replicated to 128."""
    n = ia.shape[0]
    rows_per_g = grp * 128
    parts = []
    for g in range(n // rows_per_g):
        sl = slice(g * rows_per_g, (g + 1) * rows_per_g)
        parts.append(ia[sl])
        parts.append(ib[sl])
    merged = np.concatenate(parts).astype(np.int16)  # [2n]
    blk = merged.reshape(2 * n // 16, 16).T  # [16, 2n/16]
    return np.ascontiguousarray(np.tile(blk, (8, 1)))  # [128, 2n/16]


def _softmax_coef(w: np.ndarray) -> np.ndarray:
    """rows of w [O, 16] -> coef [O, 4] = softmax(w) @ OP_COEF (float64)."""
    w = w.astype(np.float64)
    e = np.exp(w - w.max(axis=1, keepdims=True))
    p = e / e.sum(axis=1, keepdims=True)
    return p @ _OP_COEF


def _prep_shared(w1, w2, w3, idx_a1, idx_b1, idx_a2, idx_b2, idx_a3, idx_b3,
                 grp=GRP):
    """Host-side layout prep: per-layer output-row permutation sigma (sorting
    the a-gather), source-row remap pi into the partition-major stored
    layout, host-computed polynomial coefficients, L3 matmul stationaries,
    and merged wrapped indices."""
    ws = (w1, w2, w3)
    ias = (idx_a1, idx_a2, idx_a3)
    ibs = (idx_b1, idx_b2, idx_b3)

    shared = {}
    ck_parts = [[] for _ in range(4)]
    pi_prev = None  # original source row -> stored virtual row
    for li in range(3):
        o = LAYERS[li][0]
        nch = NCH[li]
        ia = ias[li].astype(np.int64)
        ib = ibs[li].astype(np.int64)
        if pi_prev is not None:
            ia = pi_prev[ia]
            ib = pi_prev[ib]
        if li < 2:
            sigma = np.argsort(ia, kind="stable")
        else:
            # keep group structure: sort within each block of 1024 rows
            sigma = np.concatenate(
                [g * 1024 + np.argsort(ia[g * 1024 : (g + 1) * 1024], kind="stable")
                 for g in range(K)]
            )
        ia_s = ia[sigma]
        ib_s = ib[sigma]
        coef = _softmax_coef(np.asarray(ws[li], np.float64)[sigma])  # [o, 4]
        # tile layout [128, nch]: row r=c*128+p -> [p, c]
        for k in range(4):
            ck_parts[k].append(
                np.ascontiguousarray(coef[:, k].reshape(nch, 128).T)
            )
        shared[f"iab{li}"] = _wrap_idx_merged(ia_s, ib_s, grp)
        if li < 2:
            inv = np.empty(o, np.int64)
            inv[sigma] = np.arange(o)
            pi_prev = (inv % 128) * nch + inv // 128

    for k in range(4):
        shared[f"ck{k}"] = np.ascontiguousarray(
            np.concatenate(ck_parts[k], axis=1).astype(np.float32)
        )

    # L3 matmul stationaries: selg = unweighted group one-hot (for the w
    # term, whose coefficients are already applied elementwise), selk2 =
    # ck2-weighted one-hot (for the c2*b term)
    nch3 = NCH[2]
    ck3slice = [shared[f"ck{k}"][:, CH_OFF[2]:] for k in range(4)]  # [128, 80]
    selg = np.zeros((128, nch3 * K), np.float16)
    sel2 = np.zeros((128, nch3 * K), np.float16)
    for c in range(nch3):
        selg[:, c * K + c // 8] = 1.0
        sel2[:, c * K + c // 8] = ck3slice[2][:, c]
    shared["selkg"] = selg
    shared["selk2"] = sel2
    # W0[p, g] = sum_{c in group g} ck0[p, c]
    shared["w0sum"] = np.ascontiguousarray(
        ck3slice[0].reshape(128, K, 8).sum(axis=2).astype(np.float16)
    )
    return shared


def make_in_maps(x, **shared_inputs):
    shared = _prep_shared(**shared_inputs)
    in_maps = []
    for c in range(NCORES):
        xs = x[c * BS : (c + 1) * BS].astype(np.float16)  # [512, 1024]
        xT = np.ascontiguousarray(xs.T)  # [1024, 512]
        in_maps.append({"xT": xT, **shared})
    return in_maps


def get_nc(repeat=1, **opts):
    key = (repeat, tuple(sorted(opts.items())))
    if key not in _nc_cache:
        _nc_cache[key] = _build_nc(repeat, **opts)
    return _nc_cache[key]


def kernel(
    x, w1, w2, w3, idx_a1, idx_b1, idx_a2, idx_b2, idx_a3, idx_b3
) -> np.ndarray:
    from concourse.bass_utils import run_bass_kernel_spmd

    nc = get_nc()
    in_maps = make_in_maps(
        np.asarray(x),
        w1=np.asarray(w1),
        w2=np.asarray(w2),
        w3=np.asarray(w3),
        idx_a1=np.asarray(idx_a1),
        idx_b1=np.asarray(idx_b1),
        idx_a2=np.asarray(idx_a2),
        idx_b2=np.asarray(idx_b2),
        idx_a3=np.asarray(idx_a3),
        idx_b3=np.asarray(idx_b3),
    )
    res = run_bass_kernel_spmd(nc, in_maps, core_ids=list(range(NCORES)))
    out = np.empty((B, K), np.float32)
    for c in range(NCORES):
        out[c * BS : (c + 1) * BS] = res.results[c]["out"].T
    return out
